# revision 20
# baseline (speedup 1.0000x reference)
"""Causal self-attention Trainium2 kernel (8-core SPMD).

Reference: y = softmax(mask(q k^T / sqrt(dh))) v -> proj, with
x [B=4, T=2048, C=1024], H=16 heads, dh=64.

Sharding: core i handles batch b = i//2 and head-group g = i%2 (8 heads).
Each core computes a partial y (its heads' contribution to the output
projection); the host sums the two partials per batch and adds proj_b.

Per-core device program (all operands pre-transposed on host so every
matmul contraction dim lands on SBUF partitions):
  phase 1: qkT[1024, T] = Wqk_loc @ x_b^T   (q rows pre-scaled by 1/8)
  phase 2: V[T, 520]    = x_b @ Wv_loc^T    (+bias; col 64 of each 65-wide
           head group is 1.0 -> PV matmul also produces softmax row-sums)
  phase 3: qb-outer / head-pair-inner. Per (qb, hp): S^T = K @ Q^T in
           PSUM (diagonal chunks column-trimmed + packed), exp on ACT,
           causal tril mask on DVE, O^T|rowsum accumulated via PV
           matmuls; normalize = DVE recip -> DMA partition-broadcast ->
           fused DVE mul into O^T.
  phase 4: y_partial[T, 1024] = O @ Wo_loc^T, emitted per query block,
           interleaved into the next block's attention stream.
"""

import numpy as np

C = 1024
HLOC = 8
DH = 64
QB = 512  # query block (PSUM bank width in fp32)
KC = 128  # key chunk (partition dim)

_cache = {}


def _build(T, mm_dt):
    import concourse.bass as bass
    import concourse.tile as tile
    from concourse import bacc, mybir

    f32 = mybir.dt.float32
    nqb = T // QB
    ctiles = C // 128
    ttiles = T // 128

    mdt = {
        "f32r": mybir.dt.float32r,
        "bf16": mybir.dt.bfloat16,
        "f32": f32,
    }[mm_dt]

    nc = bacc.Bacc("TRN2", target_bir_lowering=False, debug=False)

    xT = nc.dram_tensor("xT", [C, T], mdt, kind="ExternalInput")
    wqkT = nc.dram_tensor("wqkT", [C // 128, 128, C // 128, 128], mdt, kind="ExternalInput")
    wvT = nc.dram_tensor("wvT", [C, 512], mdt, kind="ExternalInput")
    woT = nc.dram_tensor("woT", [512, C], mdt, kind="ExternalInput")
    qkb = nc.dram_tensor("qkb", [C], f32, kind="ExternalInput")
    vb = nc.dram_tensor("vb", [512], f32, kind="ExternalInput")
    tril = nc.dram_tensor("tril", [128, 128], f32, kind="ExternalInput")
    y = nc.dram_tensor("y", [T, C], f32, kind="ExternalOutput")

    Exp = mybir.ActivationFunctionType.Exp

    with tile.TileContext(nc) as tc:
        with (
            tc.tile_pool(name="persist", bufs=1) as persist,
            tc.tile_pool(name="consts", bufs=1) as consts,
        ):
            qkT_sb = persist.tile([128, ctiles, T], mdt)
            V_sb = persist.tile([128, T // 128, HLOC * 65], mdt)
            tril_sb = consts.tile([128, 128], f32)
            tril_m = consts.tile([128, 128], mdt)
            qkb_sb = consts.tile([128, ctiles], f32)
            vb_sb = consts.tile([128, 512], f32)

            nc.sync.dma_start(tril_sb[:], tril[:])
            nc.vector.tensor_copy(tril_m[:], tril_sb[:])
            nc.sync.dma_start(qkb_sb[:], qkb.rearrange("(r p) -> p r", p=128))
            vb_ap = vb[:]
            nc.sync.dma_start(
                vb_sb[:],
                bass.AP(
                    tensor=vb_ap.tensor, offset=vb_ap.offset, ap=[[0, 128], [1, 512]]
                ),
            )
            # ones columns of V (col 64 of each head's 65-wide slot).
            # memset can't produce all dtypes; ACT copy with scale=0, bias=1
            # (input values irrelevant but must be finite -> use tril).
            v_grp = V_sb.rearrange("p t (h c) -> p t h c", c=65)
            nc.scalar.activation(
                v_grp[:, :, :, 64:65],
                tril_sb[:, 0 : (T // 128) * HLOC].rearrange(
                    "p (a b c) -> p a b c", a=T // 128, b=HLOC, c=1
                ),
                mybir.ActivationFunctionType.Copy,
                bias=1.0,
                scale=0.0,
            )

            # ---------------- phases 1+2: projections ----------------
            # warmup matmuls ramp the PE p-state while input DMAs stream
            with tc.tile_pool(name="pwarm", bufs=2, space="PSUM") as pwarm:
                for w in range(48):
                    wp = pwarm.tile([128, 128], f32, tag="wp", name=f"wp{w}")
                    nc.tensor.matmul(
                        wp[:], tril_m[:], tril_m[:], start=True, stop=True
                    )
            # one shared PSUM accumulator pool for phases 1/2/4 so PSUM
            # banks never hand off between pools (a handoff serializes the
            # next phase behind this phase's DVE drain backlog)
            pacc_cm = tc.tile_pool(name="pacc", bufs=2, space="PSUM")
            pacc = pacc_cm.__enter__()
            with (
                tc.tile_pool(name="xw", bufs=1) as xw,
                tc.tile_pool(name="wqks", bufs=2) as wqks,
            ):
                xT_sb = xw.tile([128, ctiles, T], mdt)
                wvT_sb = xw.tile([128, ctiles, 512], mdt)
                xT_r = xT.rearrange("(c p) t -> p c t", p=128)
                # chunked load (T halves) so phase 1 can start on the
                # first half while the rest streams in
                Th = T // 2
                for h in range(2):
                    for c in range(ctiles):
                        eng = (nc.sync, nc.gpsimd)[(h * ctiles + c) % 2]
                        eng.dma_start(
                            xT_sb[:, c, h * Th : (h + 1) * Th],
                            xT_r[:, c, h * Th : (h + 1) * Th],
                        )
                nc.gpsimd.dma_start(wvT_sb[:], wvT.rearrange("(c p) v -> p c v", p=128))

                for rt in range(ctiles):
                    wqk_t = wqks.tile([128, ctiles, 128], mdt)
                    nc.scalar.dma_start(wqk_t[:], wqkT[rt])
                    for nt in range(T // 512):
                        ps = pacc.tile([128, 512], f32, tag="pacc")
                        for c in range(ctiles):
                            nc.tensor.matmul(
                                ps[:],
                                wqk_t[:, c, :],
                                xT_sb[:, c, nt * 512 : (nt + 1) * 512],
                                start=(c == 0),
                                stop=(c == ctiles - 1),
                            )
                        nc.vector.tensor_scalar_add(
                            qkT_sb[:, rt, nt * 512 : (nt + 1) * 512],
                            ps[:],
                            qkb_sb[:, rt : rt + 1],
                        )

                for tt in range(ttiles):
                    ps = pacc.tile([128, 512], f32, tag="pacc")
                    for c in range(ctiles):
                        nc.tensor.matmul(
                            ps[:],
                            xT_sb[:, c, tt * 128 : (tt + 1) * 128],
                            wvT_sb[:, c, :],
                            start=(c == 0),
                            stop=(c == ctiles - 1),
                        )
                    nc.vector.tensor_add(
                        v_grp[:, tt, :, 0:64],
                        ps.rearrange("p (h c) -> p h c", c=64),
                        vb_sb.rearrange("p (h c) -> p h c", c=64),
                    )

            # ---------------- phases 3+4 ----------------
            with tc.tile_pool(name="ot", bufs=1) as ot:
                OT_sb = ot.tile([128, 4, T], mdt)
                woT_sb = ot.tile([128, 4, C], mdt)
                nc.gpsimd.dma_start(woT_sb[:], woT.rearrange("(c p) o -> p c o", p=128))

                with (
                    tc.tile_pool(name="pexp", bufs=4) as pexp,
                    tc.tile_pool(name="rsbp", bufs=4) as rsbp,
                    tc.tile_pool(name="yp", bufs=4) as yp,
                    tc.tile_pool(name="psS", bufs=2, space="PSUM") as psS,
                    tc.tile_pool(name="psO", bufs=2, space="PSUM") as psO,
                ):

                    def emit_proj(qb):
                        for tt in range(qb * 4, (qb + 1) * 4):
                            for nt in range(2):
                                ps = pacc.tile([128, 512], f32, tag="pacc")
                                for c4 in range(4):
                                    nc.tensor.matmul(
                                        ps[:],
                                        OT_sb[:, c4, tt * 128 : (tt + 1) * 128],
                                        woT_sb[:, c4, nt * 512 : (nt + 1) * 512],
                                        start=(c4 == 0),
                                        stop=(c4 == 3),
                                    )
                                yt = yp.tile([128, 512], f32)
                                nc.vector.tensor_copy(yt[:], ps[:])
                                nc.sync.dma_start(
                                    y[
                                        tt * 128 : (tt + 1) * 128,
                                        nt * 512 : (nt + 1) * 512,
                                    ],
                                    yt[:],
                                )

                    def chunk_geom(qb, kcp):
                        # chunk pair (2kcp, 2kcp+1): per chunk the leading
                        # fully-masked columns are trimmed and the valid
                        # regions packed contiguously in the S/exp tile
                        q0 = qb * 512
                        out = []
                        pos = 0
                        for kc in (2 * kcp, 2 * kcp + 1):
                            o = max(0, kc * 128 - q0)
                            w = 512 - o
                            out.append((kc, o, w, pos))
                            pos += w
                        return out, pos

                    def attention(qb, hp):
                        nkc = (qb + 1) * (QB // KC)
                        q0 = qb * 512
                        po = [
                            psO.tile([65, 512], f32, tag="po", name=f"po{hp}_{qb}_{i}")
                            for i in range(2)
                        ]
                        # software pipeline: S-matmul pairs one step
                        # ahead of exp+PV so PE never waits on ACT
                        stage = []  # (ps, lh, kcp) pending exp+PV
                        for kcp in range(nkc // 2 + 1):
                            if kcp < nkc // 2:
                                geom, _ = chunk_geom(qb, kcp)
                                for lh in range(2):
                                    b0 = 64 * lh
                                    ps = psS.tile([128, 1024], f32)
                                    for kc, o, w, pos in geom:
                                        nc.tensor.matmul(
                                            ps[:, pos : pos + w],
                                            qkT_sb[
                                                b0 : b0 + 64,
                                                4 + hp,
                                                kc * 128 : (kc + 1) * 128,
                                            ],
                                            qkT_sb[b0 : b0 + 64, hp, q0 + o : q0 + 512],
                                            start=True,
                                            stop=True,
                                        )
                                    stage.append((ps, lh, kcp))
                            if kcp > 0:
                                ready, stage = stage[:2], stage[2:]
                                for ps, lh, pk in ready:
                                    geom, wtot = chunk_geom(qb, pk)
                                    pt = pexp.tile([128, 1024], mdt)
                                    nc.scalar.activation(
                                        pt[:, 0:wtot], ps[:, 0:wtot], Exp
                                    )
                                    for kc, o, w, pos in geom:
                                        if kc * 128 >= q0:
                                            nc.vector.tensor_mul(
                                                pt[:, pos : pos + 128],
                                                pt[:, pos : pos + 128],
                                                tril_m[:],
                                            )
                                        nc.tensor.matmul(
                                            po[lh][:, o:512],
                                            V_sb[
                                                :,
                                                kc,
                                                (2 * hp + lh) * 65 : (2 * hp + lh) * 65
                                                + 65,
                                            ],
                                            pt[:, pos : pos + w],
                                            start=(kc == 0),
                                            stop=(kc == nkc - 1),
                                        )
                        # normalize. DVE op cost scales with free-dim size
                        # (partitions are parallel lanes), so reciprocal runs
                        # on a [128,4] repack of the rowsum row:
                        #   DVE copy rowsum out of PSUM -> DMA repack across
                        #   partitions -> DVE recip [128,4] -> DMA repack back
                        #   -> DMA broadcast over 64 dh partitions -> fused
                        #   DVE multiply into O^T
                        for lh in range(2):
                            b0 = 64 * lh
                            sr = rsbp.tile([1, 512], f32, tag="sr")
                            nc.vector.tensor_copy(sr[:], po[lh][64:65, :])
                            srp = rsbp.tile([128, 4], f32, tag="srp")
                            nc.gpsimd.dma_start(srp[:], sr[:])
                            nc.vector.reciprocal(srp[:], srp[:])
                            sl = rsbp.tile([1, 512], f32, tag="sl")
                            nc.gpsimd.dma_start(sl[:], srp[:])
                            rb = rsbp.tile([64, 512], f32, tag="rb")
                            src = bass.AP(
                                tensor=sl.tensor,
                                offset=sl.offset,
                                ap=[list(sl.ap[0]), [0, 64], [1, 512]],
                            )
                            nc.gpsimd.dma_start(rb[:], src)
                            nc.vector.tensor_mul(
                                OT_sb[b0 : b0 + 64, hp, q0 : q0 + 512],
                                po[lh][0:64, :],
                                rb[:],
                            )

                    for qb in range(nqb):
                        for hp in range(4):
                            attention(qb, hp)
                            if hp == 0 and qb > 0:
                                emit_proj(qb - 1)
                    emit_proj(nqb - 1)

            pacc_cm.__exit__(None, None, None)

    nc.compile()
    return nc


def get_nc(T=2048, mm_dt="bf16"):
    key = (T, mm_dt)
    if key not in _cache:
        _cache[key] = _build(T, mm_dt)
    return _cache[key]


def make_in_maps(x, qkv_w, qkv_b, proj_w, proj_b, mm_dt="bf16"):
    B, T, _ = x.shape
    f = np.float32
    if mm_dt == "bf16":
        import ml_dtypes

        md = ml_dtypes.bfloat16
    else:
        md = f
    # S^T blocks are [key, query]: keep k <= q  ->  upper triangle
    tril = np.triu(np.ones((128, 128), f))
    in_maps = []
    for i in range(B * 2):
        b, g = i // 2, i % 2
        sl = slice(g * 512, (g + 1) * 512)
        wq = qkv_w[0 * C : 1 * C][sl] * (1.0 / 8.0)
        wk = qkv_w[1 * C : 2 * C][sl]
        wv = qkv_w[2 * C : 3 * C][sl]
        in_maps.append(
            {
                "xT": np.ascontiguousarray(x[b].T).astype(md),
                "wqkT": np.ascontiguousarray(
                    np.stack(
                        [
                            np.concatenate([wq, wk], 0)
                            .T[:, rt * 128 : (rt + 1) * 128]
                            .reshape(C // 128, 128, 128)
                            .transpose(1, 0, 2)
                            for rt in range(C // 128)
                        ]
                    )
                ).astype(md),
                "wvT": np.ascontiguousarray(wv.T).astype(md),
                "woT": np.ascontiguousarray(proj_w[:, sl].T).astype(md),
                "qkb": np.concatenate(
                    [qkv_b[0 * C : 1 * C][sl] * (1.0 / 8.0), qkv_b[1 * C : 2 * C][sl]]
                ).astype(f),
                "vb": qkv_b[2 * C : 3 * C][sl].astype(f),
                "tril": tril,
            }
        )
    return in_maps


def kernel(x, qkv_w, qkv_b, proj_w, proj_b, mm_dt="bf16", trace=False, tmpdir=None):
    from concourse.bass_utils import run_bass_kernel_spmd

    x = np.asarray(x, np.float32)
    qkv_w = np.asarray(qkv_w, np.float32)
    qkv_b = np.asarray(qkv_b, np.float32)
    proj_w = np.asarray(proj_w, np.float32)
    proj_b = np.asarray(proj_b, np.float32)

    B, T, _ = x.shape
    nc = get_nc(T, mm_dt)
    in_maps = make_in_maps(x, qkv_w, qkv_b, proj_w, proj_b, mm_dt)
    res = run_bass_kernel_spmd(
        nc, in_maps, list(range(len(in_maps))), trace=trace, tmpdir=tmpdir
    )
    out = np.empty((B, T, C), np.float32)
    for b in range(B):
        out[b] = res.results[2 * b]["y"] + res.results[2 * b + 1]["y"] + proj_b
    kernel.last_result = res
    return out


# revision 25
# speedup vs baseline: 1.0400x; 1.0400x over previous
"""Causal self-attention Trainium2 kernel (8-core SPMD).

Reference: y = softmax(mask(q k^T / sqrt(dh))) v -> proj, with
x [B=4, T=2048, C=1024], H=16 heads, dh=64.

Sharding: core i handles batch b = i//2 and head-group g = i%2 (8 heads).
Each core computes a partial y (its heads' contribution to the output
projection); the host sums the two partials per batch and adds proj_b.

Per-core device program (all operands pre-transposed on host so every
matmul contraction dim lands on SBUF partitions):
  phase 1: qkT[1024, T] = Wqk_loc @ x_b^T   (q rows pre-scaled by 1/8)
  phase 2: V[T, 520]    = x_b @ Wv_loc^T    (+bias; col 64 of each 65-wide
           head group is 1.0 -> PV matmul also produces softmax row-sums)
  phase 3: qb-outer / head-pair-inner. Per (qb, hp): S^T = K @ Q^T in
           PSUM (diagonal chunks column-trimmed + packed), exp on ACT,
           causal tril mask on DVE, O^T|rowsum accumulated via PV
           matmuls; normalize = DVE recip -> DMA partition-broadcast ->
           fused DVE mul into O^T.
  phase 4: y_partial[T, 1024] = O @ Wo_loc^T, emitted per query block,
           interleaved into the next block's attention stream.
"""

import numpy as np

C = 1024
HLOC = 8
DH = 64
QB = 512  # query block (PSUM bank width in fp32)
KC = 128  # key chunk (partition dim)

_cache = {}


def _build(T, mm_dt):
    import concourse.bass as bass
    import concourse.tile as tile
    from concourse import bacc, mybir

    f32 = mybir.dt.float32
    nqb = T // QB
    ctiles = C // 128
    ttiles = T // 128

    mdt = {
        "f32r": mybir.dt.float32r,
        "bf16": mybir.dt.bfloat16,
        "f32": f32,
    }[mm_dt]

    nc = bacc.Bacc("TRN2", target_bir_lowering=False, debug=False)

    xT = nc.dram_tensor("xT", [C, T], mdt, kind="ExternalInput")
    wqkT = nc.dram_tensor("wqkT", [C // 128, 128, C // 128, 128], mdt, kind="ExternalInput")
    wvT = nc.dram_tensor("wvT", [C, 512], mdt, kind="ExternalInput")
    woT = nc.dram_tensor("woT", [512, C], mdt, kind="ExternalInput")
    qkb = nc.dram_tensor("qkb", [C], f32, kind="ExternalInput")
    vb = nc.dram_tensor("vb", [512], f32, kind="ExternalInput")
    tril = nc.dram_tensor("tril", [128, 128], f32, kind="ExternalInput")
    y = nc.dram_tensor("y", [T, C], f32, kind="ExternalOutput")

    Exp = mybir.ActivationFunctionType.Exp

    with tile.TileContext(nc) as tc:
        with (
            tc.tile_pool(name="persist", bufs=1) as persist,
            tc.tile_pool(name="consts", bufs=1) as consts,
        ):
            qkT_sb = persist.tile([128, ctiles, T], mdt)
            V_sb = persist.tile([128, T // 128, HLOC * 65], mdt)
            tril_sb = consts.tile([128, 128], f32)
            tril_m = consts.tile([128, 128], mdt)
            qkb_sb = consts.tile([128, ctiles], f32)
            vb_sb = consts.tile([128, 512], f32)

            nc.sync.dma_start(tril_sb[:], tril[:])
            nc.vector.tensor_copy(tril_m[:], tril_sb[:])
            nc.sync.dma_start(qkb_sb[:], qkb.rearrange("(r p) -> p r", p=128))
            vb_ap = vb[:]
            nc.sync.dma_start(
                vb_sb[:],
                bass.AP(
                    tensor=vb_ap.tensor, offset=vb_ap.offset, ap=[[0, 128], [1, 512]]
                ),
            )
            # ones columns of V (col 64 of each head's 65-wide slot).
            # memset can't produce all dtypes; ACT copy with scale=0, bias=1
            # (input values irrelevant but must be finite -> use tril).
            v_grp = V_sb.rearrange("p t (h c) -> p t h c", c=65)
            nc.scalar.activation(
                v_grp[:, :, :, 64:65],
                tril_sb[:, 0 : (T // 128) * HLOC].rearrange(
                    "p (a b c) -> p a b c", a=T // 128, b=HLOC, c=1
                ),
                mybir.ActivationFunctionType.Copy,
                bias=1.0,
                scale=0.0,
            )

            # ---------------- phases 1+2: projections ----------------
            # warmup matmuls ramp the PE p-state while input DMAs stream
            with tc.tile_pool(name="pwarm", bufs=2, space="PSUM") as pwarm:
                for w in range(48):
                    wp = pwarm.tile([128, 128], f32, tag="wp", name=f"wp{w}")
                    nc.tensor.matmul(
                        wp[:], tril_m[:], tril_m[:], start=True, stop=True
                    )
            # All remaining pools stay open until the end of the kernel:
            # any pool close inserts a teardown barrier that serializes the
            # next phase behind this phase's DVE drain backlog.
            from contextlib import ExitStack

            stack = ExitStack()
            pacc = stack.enter_context(tc.tile_pool(name="pacc", bufs=2, space="PSUM"))
            xw = stack.enter_context(tc.tile_pool(name="xw", bufs=1))
            wqks = stack.enter_context(tc.tile_pool(name="wqks", bufs=2))
            if True:
                xT_sb = xw.tile([128, ctiles, T], mdt)
                wvT_sb = xw.tile([128, ctiles, 512], mdt)
                xT_r = xT.rearrange("(c p) t -> p c t", p=128)
                # chunked load (T halves) so phase 1 can start on the
                # first half while the rest streams in
                Th = T // 2
                for h in range(2):
                    for c in range(ctiles):
                        eng = (nc.sync, nc.gpsimd)[(h * ctiles + c) % 2]
                        eng.dma_start(
                            xT_sb[:, c, h * Th : (h + 1) * Th],
                            xT_r[:, c, h * Th : (h + 1) * Th],
                        )
                nc.gpsimd.dma_start(wvT_sb[:], wvT.rearrange("(c p) v -> p c v", p=128))

                for rt in range(ctiles):
                    wqk_t = wqks.tile([128, ctiles, 128], mdt)
                    nc.scalar.dma_start(wqk_t[:], wqkT[rt])
                    for nt in range(T // 512):
                        ps = pacc.tile([128, 512], f32, tag="pacc")
                        for c in range(ctiles):
                            nc.tensor.matmul(
                                ps[:],
                                wqk_t[:, c, :],
                                xT_sb[:, c, nt * 512 : (nt + 1) * 512],
                                start=(c == 0),
                                stop=(c == ctiles - 1),
                            )
                        nc.vector.tensor_scalar_add(
                            qkT_sb[:, rt, nt * 512 : (nt + 1) * 512],
                            ps[:],
                            qkb_sb[:, rt : rt + 1],
                        )

                for tt in range(ttiles):
                    ps = pacc.tile([128, 512], f32, tag="pacc")
                    for c in range(ctiles):
                        nc.tensor.matmul(
                            ps[:],
                            xT_sb[:, c, tt * 128 : (tt + 1) * 128],
                            wvT_sb[:, c, :],
                            start=(c == 0),
                            stop=(c == ctiles - 1),
                        )
                    nc.vector.tensor_add(
                        v_grp[:, tt, :, 0:64],
                        ps.rearrange("p (h c) -> p h c", c=64),
                        vb_sb.rearrange("p (h c) -> p h c", c=64),
                    )

            # ---------------- phases 3+4 ----------------
            if True:
                ot = stack.enter_context(tc.tile_pool(name="ot", bufs=1))
                OT_sb = ot.tile([128, 4, T], mdt)
                woT_sb = ot.tile([128, 4, C], mdt)
                nc.gpsimd.dma_start(woT_sb[:], woT.rearrange("(c p) o -> p c o", p=128))

                pexp = stack.enter_context(tc.tile_pool(name="pexp", bufs=4))
                rsbp = stack.enter_context(tc.tile_pool(name="rsbp", bufs=4))
                yp = stack.enter_context(tc.tile_pool(name="yp", bufs=4))
                psS = stack.enter_context(tc.tile_pool(name="psS", bufs=2, space="PSUM"))
                psO = stack.enter_context(tc.tile_pool(name="psO", bufs=2, space="PSUM"))
                if True:

                    def emit_proj(qb):
                        for tt in range(qb * 4, (qb + 1) * 4):
                            for nt in range(2):
                                ps = pacc.tile([128, 512], f32, tag="pacc")
                                for c4 in range(4):
                                    nc.tensor.matmul(
                                        ps[:],
                                        OT_sb[:, c4, tt * 128 : (tt + 1) * 128],
                                        woT_sb[:, c4, nt * 512 : (nt + 1) * 512],
                                        start=(c4 == 0),
                                        stop=(c4 == 3),
                                    )
                                yt = yp.tile([128, 512], f32)
                                nc.vector.tensor_copy(yt[:], ps[:])
                                nc.sync.dma_start(
                                    y[
                                        tt * 128 : (tt + 1) * 128,
                                        nt * 512 : (nt + 1) * 512,
                                    ],
                                    yt[:],
                                )

                    def chunk_geom(qb, kcp):
                        # chunk pair (2kcp, 2kcp+1): per chunk the leading
                        # fully-masked columns are trimmed and the valid
                        # regions packed contiguously in the S/exp tile
                        q0 = qb * 512
                        out = []
                        pos = 0
                        for kc in (2 * kcp, 2 * kcp + 1):
                            o = max(0, kc * 128 - q0)
                            w = 512 - o
                            out.append((kc, o, w, pos))
                            pos += w
                        return out, pos

                    def attention(qb, hp):
                        nkc = (qb + 1) * (QB // KC)
                        q0 = qb * 512
                        po = [
                            psO.tile([65, 512], f32, tag="po", name=f"po{hp}_{qb}_{i}")
                            for i in range(2)
                        ]
                        # software pipeline: S-matmul pairs one step
                        # ahead of exp+PV so PE never waits on ACT
                        stage = []  # (ps, lh, kcp) pending exp+PV
                        for kcp in range(nkc // 2 + 1):
                            if kcp < nkc // 2:
                                geom, _ = chunk_geom(qb, kcp)
                                for lh in range(2):
                                    b0 = 64 * lh
                                    ps = psS.tile([128, 1024], f32)
                                    for kc, o, w, pos in geom:
                                        nc.tensor.matmul(
                                            ps[:, pos : pos + w],
                                            qkT_sb[
                                                b0 : b0 + 64,
                                                4 + hp,
                                                kc * 128 : (kc + 1) * 128,
                                            ],
                                            qkT_sb[b0 : b0 + 64, hp, q0 + o : q0 + 512],
                                            start=True,
                                            stop=True,
                                        )
                                    stage.append((ps, lh, kcp))
                            if kcp > 0:
                                ready, stage = stage[:2], stage[2:]
                                for ps, lh, pk in ready:
                                    geom, wtot = chunk_geom(qb, pk)
                                    pt = pexp.tile([128, 1024], mdt)
                                    nc.scalar.activation(
                                        pt[:, 0:wtot], ps[:, 0:wtot], Exp
                                    )
                                    for kc, o, w, pos in geom:
                                        if kc * 128 >= q0:
                                            nc.vector.tensor_mul(
                                                pt[:, pos : pos + 128],
                                                pt[:, pos : pos + 128],
                                                tril_m[:],
                                            )
                                        nc.tensor.matmul(
                                            po[lh][:, o:512],
                                            V_sb[
                                                :,
                                                kc,
                                                (2 * hp + lh) * 65 : (2 * hp + lh) * 65
                                                + 65,
                                            ],
                                            pt[:, pos : pos + w],
                                            start=(kc == 0),
                                            stop=(kc == nkc - 1),
                                        )
                        # normalize. DVE op cost scales with free-dim size
                        # (partitions are parallel lanes), so reciprocal runs
                        # on a [128,4] repack of the rowsum row:
                        #   DVE copy rowsum out of PSUM -> DMA repack across
                        #   partitions -> DVE recip [128,4] -> DMA repack back
                        #   -> DMA broadcast over 64 dh partitions -> fused
                        #   DVE multiply into O^T
                        # stash rowsum + unnormalized O^T first so po frees
                        # after ~1.4us of DVE copies instead of after the
                        # whole DMA chain (psO bufs=2 -> next head pair's PV
                        # would stall otherwise)
                        srs = []
                        for lh in range(2):
                            b0 = 64 * lh
                            sr = rsbp.tile([1, 512], f32, tag="sr")
                            nc.vector.tensor_copy(sr[:], po[lh][64:65, :])
                            nc.vector.tensor_copy(
                                OT_sb[b0 : b0 + 64, hp, q0 : q0 + 512],
                                po[lh][0:64, :],
                            )
                            srs.append(sr)
                        rb = rsbp.tile([128, 512], f32, tag="rb")
                        for lh in range(2):
                            b0 = 64 * lh
                            sr = srs[lh]
                            srp = rsbp.tile([128, 4], f32, tag="srp")
                            nc.gpsimd.dma_start(srp[:], sr[:])
                            nc.vector.reciprocal(srp[:], srp[:])
                            sl = rsbp.tile([1, 512], f32, tag="sl")
                            nc.gpsimd.dma_start(sl[:], srp[:])
                            src = bass.AP(
                                tensor=sl.tensor,
                                offset=sl.offset,
                                ap=[list(sl.ap[0]), [0, 64], [1, 512]],
                            )
                            nc.gpsimd.dma_start(rb[b0 : b0 + 64, :], src)
                            nc.vector.tensor_mul(
                                OT_sb[b0 : b0 + 64, hp, q0 : q0 + 512],
                                OT_sb[b0 : b0 + 64, hp, q0 : q0 + 512],
                                rb[b0 : b0 + 64, :],
                            )

                    for qb in range(nqb):
                        for hp in range(4):
                            attention(qb, hp)
                            if hp == 0 and qb > 0:
                                emit_proj(qb - 1)
                    emit_proj(nqb - 1)

            stack.close()

    nc.compile()
    return nc


def get_nc(T=2048, mm_dt="bf16"):
    key = (T, mm_dt)
    if key not in _cache:
        _cache[key] = _build(T, mm_dt)
    return _cache[key]


def make_in_maps(x, qkv_w, qkv_b, proj_w, proj_b, mm_dt="bf16"):
    B, T, _ = x.shape
    f = np.float32
    if mm_dt == "bf16":
        import ml_dtypes

        md = ml_dtypes.bfloat16
    else:
        md = f
    # S^T blocks are [key, query]: keep k <= q  ->  upper triangle
    tril = np.triu(np.ones((128, 128), f))
    in_maps = []
    for i in range(B * 2):
        b, g = i // 2, i % 2
        sl = slice(g * 512, (g + 1) * 512)
        wq = qkv_w[0 * C : 1 * C][sl] * (1.0 / 8.0)
        wk = qkv_w[1 * C : 2 * C][sl]
        wv = qkv_w[2 * C : 3 * C][sl]
        in_maps.append(
            {
                "xT": np.ascontiguousarray(x[b].T).astype(md),
                "wqkT": np.ascontiguousarray(
                    np.stack(
                        [
                            np.concatenate([wq, wk], 0)
                            .T[:, rt * 128 : (rt + 1) * 128]
                            .reshape(C // 128, 128, 128)
                            .transpose(1, 0, 2)
                            for rt in range(C // 128)
                        ]
                    )
                ).astype(md),
                "wvT": np.ascontiguousarray(wv.T).astype(md),
                "woT": np.ascontiguousarray(proj_w[:, sl].T).astype(md),
                "qkb": np.concatenate(
                    [qkv_b[0 * C : 1 * C][sl] * (1.0 / 8.0), qkv_b[1 * C : 2 * C][sl]]
                ).astype(f),
                "vb": qkv_b[2 * C : 3 * C][sl].astype(f),
                "tril": tril,
            }
        )
    return in_maps


def kernel(x, qkv_w, qkv_b, proj_w, proj_b, mm_dt="bf16", trace=False, tmpdir=None):
    from concourse.bass_utils import run_bass_kernel_spmd

    x = np.asarray(x, np.float32)
    qkv_w = np.asarray(qkv_w, np.float32)
    qkv_b = np.asarray(qkv_b, np.float32)
    proj_w = np.asarray(proj_w, np.float32)
    proj_b = np.asarray(proj_b, np.float32)

    B, T, _ = x.shape
    nc = get_nc(T, mm_dt)
    in_maps = make_in_maps(x, qkv_w, qkv_b, proj_w, proj_b, mm_dt)
    res = run_bass_kernel_spmd(
        nc, in_maps, list(range(len(in_maps))), trace=trace, tmpdir=tmpdir
    )
    out = np.empty((B, T, C), np.float32)
    for b in range(B):
        out[b] = res.results[2 * b]["y"] + res.results[2 * b + 1]["y"] + proj_b
    kernel.last_result = res
    return out


# revision 26
# speedup vs baseline: 1.4682x; 1.4117x over previous
"""Causal self-attention Trainium2 kernel (8-core SPMD).

Reference: y = softmax(mask(q k^T / sqrt(dh))) v -> proj, with
x [B=4, T=2048, C=1024], H=16 heads, dh=64.

Sharding: core i handles batch b = i//2 and head-group g = i%2 (8 heads).
Each core computes a partial y (its heads' contribution to the output
projection); the host sums the two partials per batch and adds proj_b.

Per-core device program (all operands pre-transposed on host so every
matmul contraction dim lands on SBUF partitions):
  phase 1: qkT[1024, T] = Wqk_loc @ x_b^T   (q rows pre-scaled by 1/8)
  phase 2: V[T, 520]    = x_b @ Wv_loc^T    (+bias; col 64 of each 65-wide
           head group is 1.0 -> PV matmul also produces softmax row-sums)
  phase 3: per head pair, per 512-query block: S^T = K_chunk @ Q^T in PSUM
           (diagonal chunks column-trimmed + packed so fully-masked columns
           are never computed or exp'd), exp on ACT (PSUM->SBUF), causal
           tril mask on DVE, O^T|rowsum accumulated via PV matmuls,
           normalize by DMA-broadcast 1/rowsum
  phase 4: y_partial[T, 1024] = O @ Wo_loc^T
"""

import numpy as np

C = 1024
HLOC = 8
DH = 64
QB = 512  # query block (PSUM bank width in fp32)
KC = 128  # key chunk (partition dim)

_cache = {}


def _build(T, mm_dt):
    import concourse.bass as bass
    import concourse.tile as tile
    from concourse import bacc, mybir

    f32 = mybir.dt.float32
    nqb = T // QB
    ctiles = C // 128
    ttiles = T // 128

    mdt = {
        "f32r": mybir.dt.float32r,
        "bf16": mybir.dt.bfloat16,
        "f32": f32,
    }[mm_dt]

    nc = bacc.Bacc("TRN2", target_bir_lowering=False, debug=False)

    xT = nc.dram_tensor("xT", [C, T], mdt, kind="ExternalInput")
    wqkT = nc.dram_tensor("wqkT", [C // 128, 128, C // 128, 128], mdt, kind="ExternalInput")
    wvT = nc.dram_tensor("wvT", [C, 512], mdt, kind="ExternalInput")
    woT = nc.dram_tensor("woT", [512, C], mdt, kind="ExternalInput")
    qkb = nc.dram_tensor("qkb", [C], f32, kind="ExternalInput")
    vb = nc.dram_tensor("vb", [512], f32, kind="ExternalInput")
    tril = nc.dram_tensor("tril", [128, 128], f32, kind="ExternalInput")
    y = nc.dram_tensor("y", [T, C], f32, kind="ExternalOutput")

    Exp = mybir.ActivationFunctionType.Exp

    def chunk_geom(qb, kcp):
        # chunk pair (2kcp, 2kcp+1): per chunk the leading fully-masked
        # columns are trimmed and the valid regions packed contiguously
        q0 = qb * 512
        out = []
        pos = 0
        for kc in (2 * kcp, 2 * kcp + 1):
            o = max(0, kc * 128 - q0)
            w = 512 - o
            out.append((kc, o, w, pos))
            pos += w
        return out, pos

    with tile.TileContext(nc) as tc:
        with (
            tc.tile_pool(name="persist", bufs=1) as persist,
            tc.tile_pool(name="consts", bufs=1) as consts,
        ):
            qkT_sb = persist.tile([128, ctiles, T], mdt)
            V_sb = persist.tile([128, T // 128, HLOC * 65], mdt)
            tril_sb = consts.tile([128, 128], f32)
            tril_m = consts.tile([128, 128], mdt)
            qkb_sb = consts.tile([128, ctiles], f32)
            vb_sb = consts.tile([128, 512], f32)

            nc.sync.dma_start(tril_sb[:], tril[:])
            nc.vector.tensor_copy(tril_m[:], tril_sb[:])
            nc.sync.dma_start(qkb_sb[:], qkb.rearrange("(r p) -> p r", p=128))
            vb_ap = vb[:]
            nc.sync.dma_start(
                vb_sb[:],
                bass.AP(
                    tensor=vb_ap.tensor, offset=vb_ap.offset, ap=[[0, 128], [1, 512]]
                ),
            )
            # ones columns of V (col 64 of each head's 65-wide slot).
            # memset can't produce all dtypes; ACT copy with scale=0, bias=1
            # (input values irrelevant but must be finite -> use tril).
            v_grp = V_sb.rearrange("p t (h c) -> p t h c", c=65)
            nc.scalar.activation(
                v_grp[:, :, :, 64:65],
                tril_sb[:, 0 : (T // 128) * HLOC].rearrange(
                    "p (a b c) -> p a b c", a=T // 128, b=HLOC, c=1
                ),
                mybir.ActivationFunctionType.Copy,
                bias=1.0,
                scale=0.0,
            )

            # ---------------- phases 1+2: projections ----------------
            with tc.tile_pool(name="pwarm", bufs=2, space="PSUM") as pwarm:
                for w in range(24):
                    wp = pwarm.tile([128, 512], f32, tag="wp", name=f"wp{w}")
                    nc.tensor.matmul(
                        wp[:], tril_sb[:], vb_sb[:], start=True, stop=True
                    )
            with (
                tc.tile_pool(name="xw", bufs=1) as xw,
                tc.tile_pool(name="wqks", bufs=2) as wqks,
                tc.tile_pool(name="pj", bufs=4, space="PSUM") as pj,
            ):
                xT_sb = xw.tile([128, ctiles, T], mdt)
                wvT_sb = xw.tile([128, ctiles, 512], mdt)
                xT_r = xT.rearrange("(c p) t -> p c t", p=128)
                for c in range(ctiles):
                    eng = (nc.sync, nc.gpsimd, nc.scalar)[c % 3]
                    eng.dma_start(xT_sb[:, c, :], xT_r[:, c, :])
                nc.gpsimd.dma_start(wvT_sb[:], wvT.rearrange("(c p) v -> p c v", p=128))

                for rt in range(ctiles):
                    wqk_t = wqks.tile([128, ctiles, 128], mdt)
                    nc.sync.dma_start(wqk_t[:], wqkT[rt])
                    for nt in range(T // 512):
                        ps = pj.tile([128, 512], f32)
                        for c in range(ctiles):
                            nc.tensor.matmul(
                                ps[:],
                                wqk_t[:, c, :],
                                xT_sb[:, c, nt * 512 : (nt + 1) * 512],
                                start=(c == 0),
                                stop=(c == ctiles - 1),
                            )
                        nc.vector.tensor_scalar_add(
                            qkT_sb[:, rt, nt * 512 : (nt + 1) * 512],
                            ps[:],
                            qkb_sb[:, rt : rt + 1],
                        )

                for tt in range(ttiles):
                    ps = pj.tile([128, 512], f32)
                    for c in range(ctiles):
                        nc.tensor.matmul(
                            ps[:],
                            xT_sb[:, c, tt * 128 : (tt + 1) * 128],
                            wvT_sb[:, c, :],
                            start=(c == 0),
                            stop=(c == ctiles - 1),
                        )
                    nc.vector.tensor_add(
                        v_grp[:, tt, :, 0:64],
                        ps.rearrange("p (h c) -> p h c", c=64),
                        vb_sb.rearrange("p (h c) -> p h c", c=64),
                    )

            # ---------------- phases 3+4 ----------------
            with tc.tile_pool(name="ot", bufs=1) as ot:
                OT_sb = ot.tile([128, 4, T], mdt)
                woT_sb = ot.tile([128, 4, C], mdt)
                nc.gpsimd.dma_start(woT_sb[:], woT.rearrange("(c p) o -> p c o", p=128))

                # rowsum stash: packed on partition bases {0,32,64,96}
                # (the legal DVE bases): partition 64*lh + 32*(idx%2), col idx//2
                rs_all = ot.tile([97, (4 * nqb + 1) // 2, 512], f32)
                with (
                    tc.tile_pool(name="pexp", bufs=4) as pexp,
                    tc.tile_pool(name="rsbp", bufs=2) as rsbp,
                    tc.tile_pool(name="psS", bufs=3, space="PSUM") as psS,
                    tc.tile_pool(name="psO", bufs=2, space="PSUM") as psO,
                ):
                    for hp in range(4):
                        for qb in range(nqb):
                            po = [
                                psO.tile([65, 512], f32, tag="po", name=f"po{hp}_{qb}_{i}")
                                for i in range(2)
                            ]
                            nkc = (qb + 1) * (QB // KC)
                            q0 = qb * 512
                            q_sl = slice(qb * 512, (qb + 1) * 512)

                            # software pipeline: S-matmul pairs one step
                            # ahead of exp+PV so PE never waits on ACT
                            stage = []  # (ps, lh, kcp) pending exp+PV
                            for kcp in range(nkc // 2 + 1):
                                if kcp < nkc // 2:
                                    geom, _ = chunk_geom(qb, kcp)
                                    for lh in range(2):
                                        b0 = 64 * lh
                                        ps = psS.tile([128, 1024], f32)
                                        for kc, o, w, pos in geom:
                                            nc.tensor.matmul(
                                                ps[:, pos : pos + w],
                                                qkT_sb[
                                                    b0 : b0 + 64,
                                                    4 + hp,
                                                    kc * 128 : (kc + 1) * 128,
                                                ],
                                                qkT_sb[
                                                    b0 : b0 + 64, hp, q0 + o : q0 + 512
                                                ],
                                                start=True,
                                                stop=True,
                                            )
                                        stage.append((ps, lh, kcp))
                                if kcp > 0:
                                    ready, stage = stage[:2], stage[2:]
                                    for ps, lh, pk in ready:
                                        geom, wtot = chunk_geom(qb, pk)
                                        pt = pexp.tile([128, 1024], mdt)
                                        nc.scalar.activation(
                                            pt[:, 0:wtot], ps[:, 0:wtot], Exp
                                        )
                                        for kc, o, w, pos in geom:
                                            if kc * 128 >= q0:
                                                nc.vector.tensor_mul(
                                                    pt[:, pos : pos + 128],
                                                    pt[:, pos : pos + 128],
                                                    tril_m[:],
                                                )
                                            nc.tensor.matmul(
                                                po[lh][:, o:512],
                                                V_sb[
                                                    :,
                                                    kc,
                                                    (2 * hp + lh) * 65 : (2 * hp + lh)
                                                    * 65
                                                    + 65,
                                                ],
                                                pt[:, pos : pos + w],
                                                start=(kc == 0),
                                                stop=(kc == nkc - 1),
                                            )

                            # stash rowsums + unnormalized O^T (pure DVE, no
                            # DMA in the PSUM-release path); normalization is
                            # batched after the attention loops
                            idx = hp * nqb + qb
                            for lh in range(2):
                                p0 = 64 * lh + 32 * (idx % 2)
                                nc.vector.tensor_copy(
                                    rs_all[p0 : p0 + 1, idx // 2, :],
                                    po[lh][64:65, :],
                                )
                                nc.vector.tensor_copy(
                                    OT_sb[64 * lh : 64 * lh + 64, hp, q_sl],
                                    po[lh][0:64, :],
                                )

                        # ---- per-hp batched 1/rowsum + normalize ----
                        # (overlaps with next head pair's attention)
                        rsc = rsbp.tile(
                            [128, max(nqb, 2) * 256 // 16],
                            f32,
                            tag="rsc",
                            name=f"rsc{hp}",
                        )
                        nc.vector.memset(rsc[:, :], 1.0)
                        groups = []  # (lh, par, col0, ncols)
                        for lh in range(2):
                            for par in range(2):
                                idxs = [
                                    hp * nqb + qb
                                    for qb in range(nqb)
                                    if (hp * nqb + qb) % 2 == par
                                ]
                                if idxs:
                                    groups.append(
                                        (lh, par, min(i // 2 for i in idxs), len(idxs))
                                    )
                        def _gflats(lh, par, col0, ncols):
                            p0 = 64 * lh + 32 * par
                            row = rs_all[p0 : p0 + 1, col0 : col0 + ncols, :]
                            n = ncols * 512
                            flat = bass.AP(
                                tensor=row.tensor,
                                offset=row.offset,
                                ap=[list(row.ap[0]), [1, n]],
                            )
                            rid = 32 * (2 * lh + par)
                            qr = rsc[rid : rid + 32, 0 : n // 32]
                            qflat = bass.AP(
                                tensor=qr.tensor,
                                offset=qr.offset,
                                ap=[list(qr.ap[0]), [1, n // 32]],
                            )
                            return flat, qflat
                        for grp in groups:
                            flat, qflat = _gflats(*grp)
                            nc.sync.dma_start(qflat, flat)
                        nc.vector.reciprocal(rsc[:, :], rsc[:, :])
                        for grp in groups:
                            flat, qflat = _gflats(*grp)
                            nc.sync.dma_start(flat, qflat)
                        for qb in range(nqb):
                            idx = hp * nqb + qb
                            q_sl = slice(qb * 512, (qb + 1) * 512)
                            rsb = rsbp.tile([128, 512], f32)
                            for lh in range(2):
                                p0 = 64 * lh + 32 * (idx % 2)
                                half = rs_all[p0 : p0 + 1, idx // 2, :]
                                src = bass.AP(
                                    tensor=half.tensor,
                                    offset=half.offset,
                                    ap=[list(half.ap[0]), [0, 64], [1, 512]],
                                )
                                nc.sync.dma_start(rsb[64 * lh : 64 * lh + 64, :], src)
                            for lh in range(2):
                                b0 = 64 * lh
                                nc.vector.tensor_mul(
                                    OT_sb[b0 : b0 + 64, hp, q_sl],
                                    OT_sb[b0 : b0 + 64, hp, q_sl],
                                    rsb[b0 : b0 + 64, :],
                                )

                # -------- phase 4: output projection --------
                with (
                    tc.tile_pool(name="yp", bufs=4) as yp,
                    tc.tile_pool(name="pj2", bufs=4, space="PSUM") as pj2,
                ):
                    for tt in range(ttiles):
                        for nt in range(2):
                            ps = pj2.tile([128, 512], f32)
                            for c4 in range(4):
                                nc.tensor.matmul(
                                    ps[:],
                                    OT_sb[:, c4, tt * 128 : (tt + 1) * 128],
                                    woT_sb[:, c4, nt * 512 : (nt + 1) * 512],
                                    start=(c4 == 0),
                                    stop=(c4 == 3),
                                )
                            yt = yp.tile([128, 512], f32)
                            nc.vector.tensor_copy(yt[:], ps[:])
                            nc.sync.dma_start(
                                y[tt * 128 : (tt + 1) * 128, nt * 512 : (nt + 1) * 512],
                                yt[:],
                            )

    nc.compile()
    return nc


def get_nc(T=2048, mm_dt="bf16"):
    key = (T, mm_dt)
    if key not in _cache:
        _cache[key] = _build(T, mm_dt)
    return _cache[key]


def make_in_maps(x, qkv_w, qkv_b, proj_w, proj_b, mm_dt="bf16"):
    B, T, _ = x.shape
    f = np.float32
    if mm_dt == "bf16":
        import ml_dtypes

        md = ml_dtypes.bfloat16
    else:
        md = f
    # S^T blocks are [key, query]: keep k <= q  ->  upper triangle
    tril = np.triu(np.ones((128, 128), f))
    in_maps = []
    for i in range(B * 2):
        b, g = i // 2, i % 2
        sl = slice(g * 512, (g + 1) * 512)
        wq = qkv_w[0 * C : 1 * C][sl] * (1.0 / 8.0)
        wk = qkv_w[1 * C : 2 * C][sl]
        wv = qkv_w[2 * C : 3 * C][sl]
        in_maps.append(
            {
                "xT": np.ascontiguousarray(x[b].T).astype(md),
                "wqkT": np.ascontiguousarray(
                    np.stack(
                        [
                            np.concatenate([wq, wk], 0)
                            .T[:, rt * 128 : (rt + 1) * 128]
                            .reshape(C // 128, 128, 128)
                            .transpose(1, 0, 2)
                            for rt in range(C // 128)
                        ]
                    )
                ).astype(md),
                "wvT": np.ascontiguousarray(wv.T).astype(md),
                "woT": np.ascontiguousarray(proj_w[:, sl].T).astype(md),
                "qkb": np.concatenate(
                    [qkv_b[0 * C : 1 * C][sl] * (1.0 / 8.0), qkv_b[1 * C : 2 * C][sl]]
                ).astype(f),
                "vb": qkv_b[2 * C : 3 * C][sl].astype(f),
                "tril": tril,
            }
        )
    return in_maps


def kernel(x, qkv_w, qkv_b, proj_w, proj_b, mm_dt="bf16", trace=False, tmpdir=None):
    from concourse.bass_utils import run_bass_kernel_spmd

    x = np.asarray(x, np.float32)
    qkv_w = np.asarray(qkv_w, np.float32)
    qkv_b = np.asarray(qkv_b, np.float32)
    proj_w = np.asarray(proj_w, np.float32)
    proj_b = np.asarray(proj_b, np.float32)

    B, T, _ = x.shape
    nc = get_nc(T, mm_dt)
    in_maps = make_in_maps(x, qkv_w, qkv_b, proj_w, proj_b, mm_dt)
    res = run_bass_kernel_spmd(
        nc, in_maps, list(range(len(in_maps))), trace=trace, tmpdir=tmpdir
    )
    out = np.empty((B, T, C), np.float32)
    for b in range(B):
        out[b] = res.results[2 * b]["y"] + res.results[2 * b + 1]["y"] + proj_b
    kernel.last_result = res
    return out
